# revision 19
# baseline (speedup 1.0000x reference)
"""DINO loss kernel for Trainium2 (8 NeuronCores, Bass/Tile) — v2.

Math (identical factorization to the fp32 baseline)
---------------------------------------------------
With q = log_softmax(student/ts) [Ns=1280, D=65536] and
p = softmax((teacher-center)/tt) [Nt=256, D]:

    loss = sum_{i != j} ( -sum_d p[i,d] q[j,d] ) / (Nt*Ns - Nt)
         = ( -(P.S/ts - C*sum(P)) + diag ) / (Nt*Ns - Nt)

    P[d] = teacher prob column sums          (device)
    S[d] = raw student logit column sums     (device)
    C    = sum_j logsumexp_j(x/ts)           (device partials, host log)
    diag = sum_i v_i/(ts*Z_i) - C_g          (v_i = sum_d e_t*sg, device)

v2: dtype-compressed transfers + top-8 logsumexp scan
-----------------------------------------------------
The fp32 baseline was DMA-bound at 48 MiB/core (~140us floor).  v2 ships
19.5 MiB/core: student_local as 5 fp8e4m3 chunks + 9 bf16 chunks + 4 bf16
half-chunks, student_global bf16, teacher fp8 (host-clamped to rowmax -
25*tt: PE mishandles bf16-subnormal e_t values).  Loss error ~5e-4 vs
the 2e-2 tolerance (validated in fp64 simulation): colsums and softmax
stats average the per-element rounding noise away.

ACT (1.2 GHz/col, dtype-blind) cannot exp everything under the ~57us DMA
floor, so row-logsumexp of the bf16 sl chunks is a DVE scan: split-half
bf16 max folds (2x mode, exact) -> InstMax top-8 per row -> ACT exps
just the 8 candidates/chunk.  At ts=0.1 the lse is top-few dominated;
rank-9+ within a folded chunk contributes < 1e-5 of the row sum.

Colsums pack vertically: sliding single-column (sl) / quad (sg, p) masks
route each 512-col piece's colsum into distinct PSUM *rows* of one
[128,512] bank via long start/stop accumulation chains, so each output
stream stages with ONE [128,512] DVE copy and retires with ONE Pool DMA.

Hardware quirks found on real trn2 (cost-model sim is blind to all):
  * TensorTensorReduce crashes the device (any dtype) — vhat is TT-mul
    (2x) + TT-add folds + short reduce instead.
  * TensorTensor is rejected on GPSIMD/Pool by codegen.
  * PE matmul weight reads do NOT wait for mid-kernel writers (DVE, ACT
    or DMA): weights must be written well before the consumer's position
    in the PE stream.  The p-colsum weights (wqs = quad mask * 1/Z) are
    therefore produced ~15us before the first p matmul, whose execution
    is structurally gated by the late-scheduled dense-chunk DMAs.
  * A dead PSUM bank absorbs PE keep-warm fillers (idle resets the
    clock ramp; cold-released matmuls cost 2-4x).
"""

import numpy as np
import ml_dtypes

import concourse.bass as bass
import concourse.bacc as bacc
import concourse.tile as tile
from concourse import mybir
from concourse.bass_utils import run_bass_kernel_spmd

F32 = mybir.dt.float32
BF16 = mybir.dt.bfloat16
F8 = mybir.dt.float8e4
AX = mybir.AxisListType
EXP = mybir.ActivationFunctionType.Exp
COPY = mybir.ActivationFunctionType.Copy
MAX = mybir.AluOpType.max
ADD = mybir.AluOpType.add
MUL = mybir.AluOpType.mult

NP_BF16 = ml_dtypes.bfloat16
NP_F8 = ml_dtypes.float8_e4m3

N_CORES = 8
D = 65536
N_T, N_G, N_L = 256, 256, 1024
CH = 4096                  # sl chunk cols
K_DENSE = 5                # fp8 chunks, exp'd densely on ACT
N_FULL = 9                 # bf16 scan chunks (fold3 + top8)
N_HALF = 4                 # trailing bf16 half-chunks (fold2 + top8)
DQ = D // 4                # sg/t cols after x4 row split

# DMA-arrival order.  Teacher first (longest dependency chain: exp -> Z ->
# 1/Z -> wqs weights), sg quarters early (vhat), dense fp8 chunks LATE:
# their PE matmuls precede the p-colsum matmuls in the PE stream, so the
# dense DMA arrival is the structural gate that keeps the p matmuls from
# racing the wqs weight write (PE weight reads don't wait for writers).
LOAD_ORDER = [
    ("t", 0), ("t", 1), ("t", 2), ("t", 3), ("g", 0), ("g", 1), ("s", 0),
    ("g", 2), ("s", 1), ("g", 3), ("s", 2), ("s", 3), ("s", 4), ("d", 0),
    ("s", 5), ("d", 1), ("s", 6), ("d", 2), ("s", 7), ("d", 3), ("s", 8),
    ("d", 4), ("h", 0), ("h", 1), ("h", 2), ("h", 3),
]

SL_ARRIVAL = [(k, i) for (k, i) in LOAD_ORDER if k in ("s", "d", "h")]


def _sl_piece_cols(kind, idx):
    """(global col base, n 512-windows) of an sl piece."""
    if kind == "d":
        return idx * CH, 8
    if kind == "s":
        return (K_DENSE + idx) * CH, 8
    return (K_DENSE + N_FULL) * CH + idx * 2048, 4


def _slot_map():
    """arrival-ordered window slots -> global sl col base (host decode)."""
    slots = []
    for kind, idx in SL_ARRIVAL:
        base, nw = _sl_piece_cols(kind, idx)
        for w in range(nw):
            slots.append(base + w * 512)
    assert len(slots) == 128
    return slots


def _masks_np():
    # sliding single-column mask: msl[p, c] = 1 iff c == 128
    msl = np.zeros((128, 256), np.float32)
    msl[:, 128] = 1.0
    # sliding quad mask: mq[p, c] = 1 iff c - 128 == p % 4
    mq = np.zeros((128, 260), np.float32)
    for p in range(128):
        mq[p, 128 + p % 4] = 1.0
    # Z-fold gmask: g[p, m] = 1 iff p//4 == m//4 (fold + broadcast in one mm)
    gm = (np.arange(128)[:, None] // 4 == np.arange(128)[None, :] // 4)
    return msl, mq, gm.astype(np.float32)


def build_nc(ts=0.1, tt=0.04):
    nc = bacc.Bacc()
    sl16 = nc.dram_tensor("sl16", [128, (N_FULL * CH) + N_HALF * 2048], BF16,
                          kind="ExternalInput")
    sl8 = nc.dram_tensor("sl8", [128, K_DENSE * CH], F8, kind="ExternalInput")
    sg = nc.dram_tensor("sg", [128, DQ], BF16, kind="ExternalInput")
    t = nc.dram_tensor("t", [128, DQ], F8, kind="ExternalInput")
    nbs = nc.dram_tensor("nbs", [128, 2], F32, kind="ExternalInput")

    msl_np, mq_np, gm_np = _masks_np()
    msl16_d = nc.inline_tensor(np.ascontiguousarray(msl_np.astype(NP_BF16)), name="msl16")
    msl8_d = nc.inline_tensor(np.ascontiguousarray(msl_np.astype(NP_F8)), name="msl8")
    mq16_d = nc.inline_tensor(np.ascontiguousarray(mq_np.astype(NP_BF16)), name="mq16")
    gm_d = nc.inline_tensor(np.ascontiguousarray(gm_np), name="gmf32")

    scols = nc.dram_tensor("scols", [128, 1024], F32, kind="ExternalOutput")
    gcols = nc.dram_tensor("gcols", [128, 512], F32, kind="ExternalOutput")
    pcols = nc.dram_tensor("pcols", [128, 512], F32, kind="ExternalOutput")
    w_sl = nc.dram_tensor("w_sl", [128, K_DENSE + 2], F32, kind="ExternalOutput")
    w_sg = nc.dram_tensor("w_sg", [128, 4], F32, kind="ExternalOutput")
    z_t = nc.dram_tensor("z_t", [128, 4], F32, kind="ExternalOutput")
    v_t = nc.dram_tensor("v_t", [128, 4], F32, kind="ExternalOutput")

    n_scan_units = N_FULL + N_HALF
    ncol_collect = 8 * n_scan_units  # 104

    with tile.TileContext(nc) as tc:
        with (
            tc.tile_pool(name="singles", bufs=1) as singles,
            tc.tile_pool(name="big", bufs=1) as big,
            tc.tile_pool(name="c16", bufs=3) as c16p,
            tc.tile_pool(name="c8", bufs=2) as c8p,
            tc.tile_pool(name="l1p", bufs=2) as l1p,
            tc.tile_pool(name="l2p", bufs=2) as l2p,
            tc.tile_pool(name="l3p", bufs=2) as l3p,
            tc.tile_pool(name="stage", bufs=2) as stage,
            tc.tile_pool(name="psA", bufs=1, space="PSUM") as psA,
            tc.tile_pool(name="psB", bufs=1, space="PSUM") as psB,
            tc.tile_pool(name="psG", bufs=1, space="PSUM") as psG,
            tc.tile_pool(name="psP", bufs=1, space="PSUM") as psP,
            tc.tile_pool(name="psZ", bufs=1, space="PSUM") as psZ,
            tc.tile_pool(name="psF", bufs=1, space="PSUM") as psF,
        ):
            # ---- t=0: tiny loads on the ACT HWDGE queue ----
            msl16 = singles.tile([128, 256], BF16)
            nc.scalar.dma_start(out=msl16, in_=msl16_d[:, :])
            msl8 = singles.tile([128, 256], F8)
            nc.scalar.dma_start(out=msl8, in_=msl8_d[:, :])
            mq16 = singles.tile([128, 260], BF16)
            nc.scalar.dma_start(out=mq16, in_=mq16_d[:, :])
            gm = singles.tile([128, 128], F32)
            nc.scalar.dma_start(out=gm, in_=gm_d[:, :])
            nb = singles.tile([128, 2], F32)
            nc.scalar.dma_start(out=nb, in_=nbs[:, :])
            nbs_t = nb[:, 0:1]
            ntm_t = nb[:, 1:2]

            # resident SBUF tensors
            t_sb = big.tile([128, DQ], F8)
            sg_sb = big.tile([128, DQ], BF16)
            e_t = big.tile([128, DQ], BF16)
            throw = big.tile([128, CH], BF16)      # ACT exp target (dead)
            thr4 = big.tile([128, 4], F32)
            vm = big.tile([128, CH], BF16)         # vhat mul out
            va = big.tile([128, CH // 2], BF16)
            vb = big.tile([128, CH // 4], BF16)
            vc = big.tile([128, CH // 8], BF16)
            collect = big.tile([128, ncol_collect], BF16)
            frhs = big.tile([128, 512], BF16)      # filler rhs
            nc.vector.memset(frhs, 0.0)

            # stats
            wS = big.tile([128, K_DENSE + 2], F32)
            wG = big.tile([128, 4], F32)
            zT = big.tile([128, 4], F32)
            vT = big.tile([128, 4], F32)
            zloc = big.tile([128, 1], F32)
            rz = big.tile([128, 1], F32)
            wqs = big.tile([128, 260], BF16)

            # PSUM banks
            bankA = psA.tile([128, 512], F32)
            bankB = psB.tile([128, 512], F32)
            bankG = psG.tile([128, 512], F32)
            bankP = psP.tile([128, 512], F32)
            bankZ = psZ.tile([128, 1], F32)
            bankF = psF.tile([128, 512], F32)      # filler sink, never read

            def fill_pe(n):
                for _ in range(n):
                    nc.tensor.matmul(bankF, msl16[:, 0:128], frhs,
                                     start=True, stop=True,
                                     skip_group_check=True)

            # ---- sl window colsums (vertical packing) ----
            slot_ctr = [0]

            def sl_piece_mm(ch_tile, kind):
                mask = msl8 if kind == "d" else msl16
                nw = ch_tile.shape[-1] // 512
                for w in range(nw):
                    s = slot_ctr[0]
                    bank = bankA if s < 64 else bankB
                    r = s % 64
                    nc.tensor.matmul(
                        bank, mask[:, (128 - r):(256 - r)],
                        ch_tile[:, w * 512:(w + 1) * 512],
                        start=(r == 0), stop=(r == 63),
                        skip_group_check=True,
                    )
                    slot_ctr[0] += 1

            # ---- scan unit: fold + top8 into collect ----
            unit_ctr = [0]

            def scan_unit(ch_tile, half):
                u = unit_ctr[0]
                if not half:
                    a1 = l1p.tile([128, 2048], BF16, tag="l1")
                    nc.vector.tensor_tensor(out=a1, in0=ch_tile[:, 0:2048],
                                            in1=ch_tile[:, 2048:4096], op=MAX)
                else:
                    a1 = ch_tile
                a2 = l2p.tile([128, 1024], BF16, tag="l2")
                nc.vector.tensor_tensor(out=a2, in0=a1[:, 0:1024],
                                        in1=a1[:, 1024:2048], op=MAX)
                a3 = l3p.tile([128, 512], BF16, tag="l3")
                nc.vector.tensor_tensor(out=a3, in0=a2[:, 0:512],
                                        in1=a2[:, 512:1024], op=MAX)
                nc.vector.max(collect[:, 8 * u:8 * u + 8], a3)
                unit_ctr[0] += 1

            # ---- vhat quarter: mul + 3 fold-adds + short reduce ----
            def vhat_q(q):
                lo = q * CH
                nc.vector.tensor_tensor(out=vm, in0=e_t[:, lo:lo + CH],
                                        in1=sg_sb[:, lo:lo + CH], op=MUL)
                nc.vector.tensor_tensor(out=va, in0=vm[:, 0:2048],
                                        in1=vm[:, 2048:4096], op=ADD)
                nc.vector.tensor_tensor(out=vb, in0=va[:, 0:1024],
                                        in1=va[:, 1024:2048], op=ADD)
                nc.vector.tensor_tensor(out=vc, in0=vb[:, 0:512],
                                        in1=vb[:, 512:1024], op=ADD)
                nc.vector.reduce_sum(vT[:, q:q + 1], vc, axis=AX.X)

            def t_exp(q):
                nc.scalar.activation(
                    e_t[:, q * CH:(q + 1) * CH], t_sb[:, q * CH:(q + 1) * CH],
                    EXP, bias=ntm_t, scale=1.0 / tt,
                    accum_out=zT[:, q:q + 1],
                )

            def sg_exp(q):
                nc.scalar.activation(
                    throw, sg_sb[:, q * CH:(q + 1) * CH],
                    EXP, bias=nbs_t, scale=1.0 / ts,
                    accum_out=wG[:, q:q + 1],
                )

            def dense_exp(ch_tile, k):
                nc.scalar.activation(
                    throw, ch_tile, EXP, bias=nbs_t, scale=1.0 / ts,
                    accum_out=wS[:, k:k + 1],
                )

            def sg_mm(j):
                nc.tensor.matmul(
                    bankG, mq16[:, (128 - 4 * j):(256 - 4 * j)],
                    sg_sb[:, j * 512:(j + 1) * 512],
                    start=(j == 0), stop=(j == 31), skip_group_check=True,
                )

            def p_mm(j):
                nc.tensor.matmul(
                    bankP, wqs[:, (128 - 4 * j):(256 - 4 * j)],
                    e_t[:, j * 512:(j + 1) * 512],
                    start=(j == 0), stop=(j == 31), skip_group_check=True,
                )

            # ================= the weave =================
            sg_pieces_done = [0]
            p_pieces_done = [0]

            def load(kind, idx):
                if kind == "t":
                    nc.sync.dma_start(
                        out=t_sb[:, idx * CH:(idx + 1) * CH],
                        in_=t[:, idx * CH:(idx + 1) * CH])
                elif kind == "g":
                    nc.sync.dma_start(
                        out=sg_sb[:, idx * CH:(idx + 1) * CH],
                        in_=sg[:, idx * CH:(idx + 1) * CH])
                elif kind == "d":
                    ch = c8p.tile([128, CH], F8, tag="c8")
                    nc.sync.dma_start(out=ch, in_=sl8[:, idx * CH:(idx + 1) * CH])
                    return ch
                elif kind == "s":
                    ch = c16p.tile([128, CH], BF16, tag="c16")
                    nc.sync.dma_start(out=ch, in_=sl16[:, idx * CH:(idx + 1) * CH])
                    return ch
                else:  # half
                    ch = c16p.tile([128, 2048], BF16, tag="ch2")
                    lo = N_FULL * CH + idx * 2048
                    nc.sync.dma_start(out=ch, in_=sl16[:, lo:lo + 2048])
                    return ch
                return None

            def emit_sg_mms(n):
                for _ in range(n):
                    j = sg_pieces_done[0]
                    if j < 32:
                        sg_mm(j)
                        sg_pieces_done[0] += 1

            def emit_p_mms(n):
                for _ in range(n):
                    j = p_pieces_done[0]
                    if j < 32:
                        p_mm(j)
                        p_pieces_done[0] += 1

            scan_seen = 0
            g_staged = [False]
            fill_pe(6)  # warm PE while first loads land
            for kind, idx in LOAD_ORDER:
                ch = load(kind, idx)
                if kind == "t":
                    t_exp(idx)
                    if idx == 3:
                        # zloc = row-sums of zT on ACT (keeps DVE free);
                        # in-stream right after the t3 exp that completes zT
                        nc.scalar.activation(thr4, zT, COPY, bias=0.0,
                                             scale=1.0, accum_out=zloc)
                elif kind == "g":
                    vhat_q(idx)
                    emit_sg_mms(8)
                    sg_exp(idx)
                elif kind in ("s", "d", "h"):
                    fill_pe(3)
                    sl_piece_mm(ch, kind)
                    if kind == "s" and idx == 0:
                        # Z fold -> 1/Z -> p-colsum weights.  The matmul sits
                        # after s0's sl matmuls in the PE stream (zloc ready
                        # by then); the DVE recip/scale follow it in EMISSION
                        # order (deps are built from program order).  wqs is
                        # thus written ~15us before the first p matmul can
                        # execute (PE weight reads race mid-kernel writers).
                        nc.tensor.matmul(bankZ, gm, zloc, start=True,
                                         stop=True, skip_group_check=True)
                        fill_pe(3)
                        nc.vector.reciprocal(rz, bankZ)
                        nc.vector.tensor_scalar_mul(wqs, mq16, rz)
                        nc.scalar.dma_start(out=z_t[:, :], in_=zT)
                    if kind == "d":
                        dense_exp(ch, idx)
                        emit_p_mms(7)
                    else:
                        scan_unit(ch, half=(kind == "h"))
                        scan_seen += 1
                    if scan_seen == 5 and kind == "s":
                        nc.scalar.activation(
                            throw[:, 0:40], collect[:, 0:40], EXP,
                            bias=nbs_t, scale=1.0 / ts,
                            accum_out=wS[:, K_DENSE:K_DENSE + 1])
                    if slot_ctr[0] == 64:
                        stA = stage.tile([128, 512], F32, tag="st")
                        nc.vector.tensor_copy(out=stA, in_=bankA)
                        nc.gpsimd.dma_start(out=scols[:, 0:512], in_=stA)
                    if sg_pieces_done[0] == 32 and not g_staged[0]:
                        g_staged[0] = True
                        stG = stage.tile([128, 512], F32, tag="st")
                        nc.vector.tensor_copy(out=stG, in_=bankG)
                        nc.gpsimd.dma_start(out=gcols[:, :], in_=stG)
                        nc.gpsimd.dma_start(out=w_sg[:, :], in_=wG)

            emit_p_mms(32)
            nc.gpsimd.dma_start(out=v_t[:, :], in_=vT)

            # W batch 2 + tails
            nc.scalar.activation(
                throw[:, 0:ncol_collect - 40], collect[:, 40:ncol_collect],
                EXP, bias=nbs_t, scale=1.0 / ts,
                accum_out=wS[:, K_DENSE + 1:K_DENSE + 2])
            nc.scalar.dma_start(out=w_sl[:, :], in_=wS)

            stB = stage.tile([128, 512], F32, tag="st")
            nc.vector.tensor_copy(out=stB, in_=bankB)
            nc.gpsimd.dma_start(out=scols[:, 512:1024], in_=stB)
            stP = stage.tile([128, 512], F32, tag="st")
            nc.vector.tensor_copy(out=stP, in_=bankP)
            nc.gpsimd.dma_start(out=pcols[:, :], in_=stP)

    nc.compile()
    return nc


_NC_CACHE = {}


def _get_nc(ts, tt):
    key = (round(ts, 9), round(tt, 9))
    if key not in _NC_CACHE:
        _NC_CACHE[key] = build_nc(ts=ts, tt=tt)
    return _NC_CACHE[key]


def _merge(results, ts, bs_scaled):
    """Host-side exact merge of per-core device outputs (float64)."""
    slots = _slot_map()
    S = np.zeros(D, np.float64)
    P = np.zeros(D, np.float64)
    C = 0.0
    C_g = 0.0
    diag1 = 0.0
    healthy = True
    for r in results:
        sc = r["scols"].astype(np.float64)     # [128, 1024]
        for s, base in enumerate(slots):
            col = sc[s, 0:512] if s < 64 else sc[s - 64, 512:1024]
            S[base:base + 512] += col
        gc = r["gcols"].astype(np.float64)     # [128, 512]
        pc = r["pcols"].astype(np.float64)
        for j in range(32):
            for q in range(4):
                lo = q * DQ + j * 512
                S[lo:lo + 512] += gc[4 * j + q]
                P[lo:lo + 512] += pc[4 * j + q]
        w = r["w_sl"].astype(np.float64)
        wsum = w.sum(axis=1)
        healthy &= bool(np.isfinite(w).all() and (wsum > 0).all())
        C += (bs_scaled + np.log(np.maximum(wsum, 1e-300))).sum()
        wg = r["w_sg"].astype(np.float64)
        healthy &= bool(np.isfinite(wg).all() and (wg.sum(axis=1) > 0).all())
        lp = (bs_scaled + np.log(np.maximum(wg.sum(axis=1), 1e-300))).reshape(32, 4)
        mxg = lp.max(axis=1, keepdims=True)
        lse_g = mxg[:, 0] + np.log(np.exp(lp - mxg).sum(axis=1))
        C += lse_g.sum()
        C_g += lse_g.sum()
        v = r["v_t"].astype(np.float64).sum(axis=1).reshape(32, 4).sum(axis=1)
        z = r["z_t"].astype(np.float64).sum(axis=1).reshape(32, 4).sum(axis=1)
        healthy &= bool(np.isfinite(v).all() and np.isfinite(z).all()
                        and (z > 0).all())
        diag1 += (v / np.maximum(z, 1e-300)).sum() / ts
        healthy &= bool(np.isfinite(r["scols"]).all()
                        and np.isfinite(r["gcols"]).all()
                        and np.isfinite(r["pcols"]).all())

    cross = P @ S / ts - C * P.sum()
    diag = diag1 - C_g
    total = -cross + diag
    n_s = N_G + N_L
    n_loss_terms = N_T * n_s - min(N_T, n_s)
    loss = total / n_loss_terms
    healthy &= bool(np.isfinite(loss))
    return loss, healthy


def _numpy_loss(sg_full, sl_full, teacher, ts, tt):
    """Exact host fallback (never hit for sane input distributions)."""
    x = np.concatenate([sg_full, sl_full], axis=0).astype(np.float64) / ts
    lq = x - x.max(axis=1, keepdims=True)
    lq -= np.log(np.exp(lq).sum(axis=1, keepdims=True))
    y = teacher.astype(np.float64) / tt
    e = np.exp(y - y.max(axis=1, keepdims=True))
    p = e / e.sum(axis=1, keepdims=True)
    ce = -(p @ lq.T)
    n_t, n_s = ce.shape
    idx = np.arange(n_t)
    ce[idx, idx] = 0.0
    return ce.sum() / (n_t * n_s - min(n_t, n_s))


def kernel(out_student_global, out_student_local, out_teacher, center,
           temp_student, temp_teacher, cent_rate_m):
    out_student_global = np.asarray(out_student_global)
    out_student_local = np.asarray(out_student_local)
    out_teacher = np.asarray(out_teacher)
    center = np.asarray(center)
    ts = float(np.asarray(temp_student).reshape(-1)[0])
    tt = float(np.asarray(temp_teacher).reshape(-1)[0])

    teacher = out_teacher
    if np.any(center):
        teacher = out_teacher - center.reshape(1, -1).astype(np.float32)
    teacher = np.ascontiguousarray(teacher, dtype=np.float32)
    sg_full = np.ascontiguousarray(out_student_global, dtype=np.float32)
    sl_full = np.ascontiguousarray(out_student_local, dtype=np.float32)

    # safe exp bound for student rows: strided-sample max + margin
    smax = max(float(sl_full.ravel()[::257].max()),
               float(sg_full.ravel()[::257].max()))
    b_s = smax + 1.0
    nbs_col = np.full((128, 1), -b_s / ts, np.float32)
    # exact teacher row maxes (device exp bias); clamp so every bf16 e_t
    # value stays in the normal range (PE mishandles bf16 subnormals);
    # adds < 65536*e^-25 ~ 1e-6 relative spurious mass to Z
    tmax = teacher.max(axis=1)
    teacher = np.maximum(teacher, (tmax - 25.0 * tt)[:, None])

    nc = _get_nc(ts, tt)
    T_ROWS = N_T // N_CORES
    SG_ROWS = N_G // N_CORES
    SL_ROWS = N_L // N_CORES
    split = K_DENSE * CH
    in_maps = []
    for c in range(N_CORES):
        slc = sl_full[c * SL_ROWS:(c + 1) * SL_ROWS]
        ntm_c = (-np.repeat(tmax[c * T_ROWS:(c + 1) * T_ROWS], 4)
                 .reshape(128, 1) / tt).astype(np.float32)
        in_maps.append({
            "sl8": np.ascontiguousarray(slc[:, :split]).astype(NP_F8),
            "sl16": np.ascontiguousarray(slc[:, split:]).astype(NP_BF16),
            "sg": sg_full[c * SG_ROWS:(c + 1) * SG_ROWS]
                  .reshape(128, DQ).astype(NP_BF16),
            "t": teacher[c * T_ROWS:(c + 1) * T_ROWS]
                 .reshape(128, DQ).astype(NP_F8),
            "nbs": np.ascontiguousarray(
                np.concatenate([nbs_col, ntm_c], axis=1)).astype(np.float32),
        })
    res = run_bass_kernel_spmd(nc, in_maps, core_ids=list(range(N_CORES)))
    loss, healthy = _merge(res.results, ts, b_s / ts)
    if not healthy:
        loss = _numpy_loss(sg_full, sl_full, teacher, ts, tt)
    return np.float32(loss)


# revision 23
# speedup vs baseline: 1.2082x; 1.2082x over previous
"""DINO loss kernel for Trainium2 (8 NeuronCores, Bass/Tile) — v2.

Math (identical factorization to the fp32 baseline)
---------------------------------------------------
With q = log_softmax(student/ts) [Ns=1280, D=65536] and
p = softmax((teacher-center)/tt) [Nt=256, D]:

    loss = sum_{i != j} ( -sum_d p[i,d] q[j,d] ) / (Nt*Ns - Nt)
         = ( -(P.S/ts - C*sum(P)) + diag ) / (Nt*Ns - Nt)

    P[d] = teacher prob column sums          (device)
    S[d] = raw student logit column sums     (device)
    C    = sum_j logsumexp_j(x/ts)           (device partials, host log)
    diag = sum_i v_i/(ts*Z_i) - C_g          (v_i = sum_d e_t*sg, device)

v2: dtype-compressed transfers + top-8 logsumexp scan
-----------------------------------------------------
The fp32 baseline was DMA-bound at 48 MiB/core (~140us floor).  v2 ships
19.5 MiB/core: student_local as 5 fp8e4m3 chunks + 9 bf16 chunks + 4 bf16
half-chunks, student_global bf16, teacher fp8 (host-clamped to rowmax -
25*tt: PE mishandles bf16-subnormal e_t values).  Loss error ~5e-4 vs
the 2e-2 tolerance (validated in fp64 simulation): colsums and softmax
stats average the per-element rounding noise away.

ACT (1.2 GHz/col, dtype-blind) cannot exp everything under the ~57us DMA
floor, so row-logsumexp of the bf16 sl chunks is a DVE scan: split-half
bf16 max folds (2x mode, exact) -> InstMax top-8 per row -> ACT exps
just the 8 candidates/chunk.  At ts=0.1 the lse is top-few dominated;
rank-9+ within a folded chunk contributes < 1e-5 of the row sum.

Colsums pack vertically: sliding single-column (sl) / quad (sg, p) masks
route each 512-col piece's colsum into distinct PSUM *rows* of one
[128,512] bank via long start/stop accumulation chains, so each output
stream stages with ONE [128,512] DVE copy and retires with ONE Pool DMA.

Hardware quirks found on real trn2 (cost-model sim is blind to all):
  * TensorTensorReduce crashes the device (any dtype) — vhat is TT-mul
    (2x) + TT-add folds + short reduce instead.
  * TensorTensor is rejected on GPSIMD/Pool by codegen.
  * PE matmul weight reads do NOT wait for mid-kernel writers (DVE, ACT
    or DMA): weights must be written well before the consumer's position
    in the PE stream.  The p-colsum weights (wqs = quad mask * 1/Z) are
    therefore produced ~15us before the first p matmul, whose execution
    is structurally gated by the late-scheduled dense-chunk DMAs.
  * A dead PSUM bank absorbs PE keep-warm fillers (idle resets the
    clock ramp; cold-released matmuls cost 2-4x).
"""

import numpy as np
import ml_dtypes

import concourse.bass as bass
import concourse.bacc as bacc
import concourse.tile as tile
from concourse import mybir
from concourse.bass_utils import run_bass_kernel_spmd

F32 = mybir.dt.float32
BF16 = mybir.dt.bfloat16
F8 = mybir.dt.float8e4
AX = mybir.AxisListType
EXP = mybir.ActivationFunctionType.Exp
COPY = mybir.ActivationFunctionType.Copy
MAX = mybir.AluOpType.max
ADD = mybir.AluOpType.add
MUL = mybir.AluOpType.mult

NP_BF16 = ml_dtypes.bfloat16
NP_F8 = ml_dtypes.float8_e4m3

N_CORES = 8
D = 65536
N_T, N_G, N_L = 256, 256, 1024
CH = 4096                  # sl chunk cols
K_DENSE = 5                # fp8 chunks, exp'd densely on ACT
N_FULL = 9                 # bf16 scan chunks (fold3 + top8)
N_HALF = 4                 # trailing bf16 half-chunks (fold2 + top8)
DQ = D // 4                # sg/t cols after x4 row split

# DMA-arrival order.  Teacher first (longest dependency chain: exp -> Z ->
# 1/Z -> wqs weights), sg quarters early (vhat), dense fp8 chunks LATE:
# their PE matmuls precede the p-colsum matmuls in the PE stream, so the
# dense DMA arrival is the structural gate that keeps the p matmuls from
# racing the wqs weight write (PE weight reads don't wait for writers).
LOAD_ORDER = [
    ("t", 0), ("t", 1), ("t", 2), ("t", 3), ("g", 0), ("g", 1), ("s", 0),
    ("g", 2), ("s", 1), ("g", 3), ("s", 2), ("s", 3), ("s", 4), ("d", 0),
    ("s", 5), ("d", 1), ("s", 6), ("d", 2), ("s", 7), ("d", 3), ("s", 8),
    ("d", 4), ("h", 0), ("h", 1), ("h", 2), ("h", 3),
]

SL_ARRIVAL = [(k, i) for (k, i) in LOAD_ORDER if k in ("s", "d", "h")]


def _sl_piece_cols(kind, idx):
    """(global col base, n 512-windows) of an sl piece."""
    if kind == "d":
        return idx * CH, 8
    if kind == "s":
        return (K_DENSE + idx) * CH, 8
    return (K_DENSE + N_FULL) * CH + idx * 2048, 4


def _slot_map():
    """arrival-ordered window slots -> global sl col base (host decode)."""
    slots = []
    for kind, idx in SL_ARRIVAL:
        base, nw = _sl_piece_cols(kind, idx)
        for w in range(nw):
            slots.append(base + w * 512)
    assert len(slots) == 128
    return slots


def _masks_np():
    # sliding single-column mask: msl[p, c] = 1 iff c == 128
    msl = np.zeros((128, 256), np.float32)
    msl[:, 128] = 1.0
    # sliding quad mask: mq[p, c] = 1 iff c - 128 == p % 4
    mq = np.zeros((128, 260), np.float32)
    for p in range(128):
        mq[p, 128 + p % 4] = 1.0
    # Z-fold gmask: g[p, m] = 1 iff p//4 == m//4 (fold + broadcast in one mm)
    gm = (np.arange(128)[:, None] // 4 == np.arange(128)[None, :] // 4)
    return msl, mq, gm.astype(np.float32)


def build_nc(ts=0.1, tt=0.04, FILL=0, WARM=8):
    nc = bacc.Bacc()
    sl16 = nc.dram_tensor("sl16", [128, (N_FULL * CH) + N_HALF * 2048], BF16,
                          kind="ExternalInput")
    sl8 = nc.dram_tensor("sl8", [128, K_DENSE * CH], F8, kind="ExternalInput")
    sg = nc.dram_tensor("sg", [128, DQ], BF16, kind="ExternalInput")
    t = nc.dram_tensor("t", [128, DQ], F8, kind="ExternalInput")
    nbs = nc.dram_tensor("nbs", [128, 2], F32, kind="ExternalInput")

    msl_np, mq_np, gm_np = _masks_np()
    msl16_d = nc.inline_tensor(np.ascontiguousarray(msl_np.astype(NP_BF16)), name="msl16")
    msl8_d = nc.inline_tensor(np.ascontiguousarray(msl_np.astype(NP_F8)), name="msl8")
    mq16_d = nc.inline_tensor(np.ascontiguousarray(mq_np.astype(NP_BF16)), name="mq16")
    gm_d = nc.inline_tensor(np.ascontiguousarray(gm_np), name="gmf32")

    scols = nc.dram_tensor("scols", [128, 1024], F32, kind="ExternalOutput")
    gcols = nc.dram_tensor("gcols", [128, 512], F32, kind="ExternalOutput")
    pcols = nc.dram_tensor("pcols", [128, 512], F32, kind="ExternalOutput")
    w_sl = nc.dram_tensor("w_sl", [128, K_DENSE + 2], F32, kind="ExternalOutput")
    w_sg = nc.dram_tensor("w_sg", [128, 4], F32, kind="ExternalOutput")
    z_t = nc.dram_tensor("z_t", [128, 4], F32, kind="ExternalOutput")
    v_t = nc.dram_tensor("v_t", [128, 4], F32, kind="ExternalOutput")

    n_scan_units = N_FULL + N_HALF
    ncol_collect = 8 * n_scan_units  # 104

    with tile.TileContext(nc) as tc:
        with (
            tc.tile_pool(name="singles", bufs=1) as singles,
            tc.tile_pool(name="big", bufs=1) as big,
            tc.tile_pool(name="c16", bufs=3) as c16p,
            tc.tile_pool(name="c8", bufs=2) as c8p,
            tc.tile_pool(name="l1p", bufs=2) as l1p,
            tc.tile_pool(name="l2p", bufs=2) as l2p,
            tc.tile_pool(name="l3p", bufs=2) as l3p,
            tc.tile_pool(name="stage", bufs=2) as stage,
            tc.tile_pool(name="psA", bufs=1, space="PSUM") as psA,
            tc.tile_pool(name="psB", bufs=1, space="PSUM") as psB,
            tc.tile_pool(name="psG", bufs=1, space="PSUM") as psG,
            tc.tile_pool(name="psP", bufs=1, space="PSUM") as psP,
            tc.tile_pool(name="psZ", bufs=1, space="PSUM") as psZ,
            tc.tile_pool(name="psF", bufs=1, space="PSUM") as psF,
        ):
            # ---- t=0: tiny loads on the ACT HWDGE queue ----
            msl16 = singles.tile([128, 256], BF16)
            nc.scalar.dma_start(out=msl16, in_=msl16_d[:, :])
            msl8 = singles.tile([128, 256], F8)
            nc.scalar.dma_start(out=msl8, in_=msl8_d[:, :])
            mq16 = singles.tile([128, 260], BF16)
            nc.scalar.dma_start(out=mq16, in_=mq16_d[:, :])
            gm = singles.tile([128, 128], F32)
            nc.scalar.dma_start(out=gm, in_=gm_d[:, :])
            nb = singles.tile([128, 2], F32)
            nc.scalar.dma_start(out=nb, in_=nbs[:, :])
            nbs_t = nb[:, 0:1]
            ntm_t = nb[:, 1:2]

            # resident SBUF tensors
            t_sb = big.tile([128, DQ], F8)
            sg_sb = big.tile([128, DQ], BF16)
            e_t = big.tile([128, DQ], BF16)
            throw = big.tile([128, 2 * CH], BF16)  # ACT exp target (dead)
            thr4 = big.tile([128, 4], F32)
            vm = big.tile([128, CH], BF16)         # vhat mul out
            va = big.tile([128, CH // 2], BF16)
            vb = big.tile([128, CH // 4], BF16)
            vc = big.tile([128, CH // 8], BF16)
            collect = big.tile([128, ncol_collect], BF16)
            frhs = big.tile([128, 512], BF16)      # filler rhs
            nc.vector.memset(frhs, 0.0)

            # stats
            wS = big.tile([128, K_DENSE + 2], F32)
            wG = big.tile([128, 4], F32)
            zT = big.tile([128, 4], F32)
            vT = big.tile([128, 4], F32)
            zloc = big.tile([128, 1], F32)
            rz = big.tile([128, 1], F32)
            wqs = big.tile([128, 260], BF16)

            # PSUM banks
            bankA = psA.tile([128, 512], F32)
            bankB = psB.tile([128, 512], F32)
            bankG = psG.tile([128, 512], F32)
            bankP = psP.tile([128, 512], F32)
            bankZ = psZ.tile([128, 1], F32)
            bankF = psF.tile([128, 512], F32)      # filler sink, never read

            def fill_pe(n):
                for _ in range(n):
                    nc.tensor.matmul(bankF, msl16[:, 0:128], frhs,
                                     start=True, stop=True,
                                     skip_group_check=True)

            # ---- sl window colsums (vertical packing) ----
            slot_ctr = [0]

            def sl_piece_mm(ch_tile, kind):
                mask = msl8 if kind == "d" else msl16
                nw = ch_tile.shape[-1] // 512
                for w in range(nw):
                    s = slot_ctr[0]
                    bank = bankA if s < 64 else bankB
                    r = s % 64
                    nc.tensor.matmul(
                        bank, mask[:, (128 - r):(256 - r)],
                        ch_tile[:, w * 512:(w + 1) * 512],
                        start=(r == 0), stop=(r == 63),
                        skip_group_check=True,
                    )
                    slot_ctr[0] += 1

            # ---- scan unit: fold + top8 into collect ----
            unit_ctr = [0]

            def scan_unit(ch_tile, half):
                u = unit_ctr[0]
                if not half:
                    a1 = l1p.tile([128, 2048], BF16, tag="l1")
                    nc.vector.tensor_tensor(out=a1, in0=ch_tile[:, 0:2048],
                                            in1=ch_tile[:, 2048:4096], op=MAX)
                else:
                    a1 = ch_tile
                a2 = l2p.tile([128, 1024], BF16, tag="l2")
                nc.vector.tensor_tensor(out=a2, in0=a1[:, 0:1024],
                                        in1=a1[:, 1024:2048], op=MAX)
                a3 = l3p.tile([128, 512], BF16, tag="l3")
                nc.vector.tensor_tensor(out=a3, in0=a2[:, 0:512],
                                        in1=a2[:, 512:1024], op=MAX)
                nc.vector.max(collect[:, 8 * u:8 * u + 8], a3)
                unit_ctr[0] += 1

            # ---- vhat quarter: mul + 3 fold-adds + short reduce ----
            def vhat_q(q):
                lo = q * CH
                nc.vector.tensor_tensor(out=vm, in0=e_t[:, lo:lo + CH],
                                        in1=sg_sb[:, lo:lo + CH], op=MUL)
                nc.vector.tensor_tensor(out=va, in0=vm[:, 0:2048],
                                        in1=vm[:, 2048:4096], op=ADD)
                nc.vector.tensor_tensor(out=vb, in0=va[:, 0:1024],
                                        in1=va[:, 1024:2048], op=ADD)
                nc.vector.tensor_tensor(out=vc, in0=vb[:, 0:512],
                                        in1=vb[:, 512:1024], op=ADD)
                nc.vector.reduce_sum(vT[:, q:q + 1], vc, axis=AX.X)

            def t_exp(q):
                nc.scalar.activation(
                    e_t[:, q * CH:(q + 1) * CH], t_sb[:, q * CH:(q + 1) * CH],
                    EXP, bias=ntm_t, scale=1.0 / tt,
                    accum_out=zT[:, q:q + 1],
                )

            def sg_exp(q):
                nc.scalar.activation(
                    throw[:, 0:CH], sg_sb[:, q * CH:(q + 1) * CH],
                    EXP, bias=nbs_t, scale=1.0 / ts,
                    accum_out=wG[:, q:q + 1],
                )

            def dense_exp(ch_tile, k):
                nc.scalar.activation(
                    throw[:, 0:CH], ch_tile, EXP, bias=nbs_t, scale=1.0 / ts,
                    accum_out=wS[:, k:k + 1],
                )

            def sg_mm(j):
                nc.tensor.matmul(
                    bankG, mq16[:, (128 - 4 * j):(256 - 4 * j)],
                    sg_sb[:, j * 512:(j + 1) * 512],
                    start=(j == 0), stop=(j == 31), skip_group_check=True,
                )

            def p_mm(j):
                nc.tensor.matmul(
                    bankP, wqs[:, (128 - 4 * j):(256 - 4 * j)],
                    e_t[:, j * 512:(j + 1) * 512],
                    start=(j == 0), stop=(j == 31), skip_group_check=True,
                )

            # ================= the weave =================
            sg_pieces_done = [0]
            p_pieces_done = [0]

            def load(kind, idx):
                if kind == "t":
                    nc.sync.dma_start(
                        out=t_sb[:, idx * CH:(idx + 1) * CH],
                        in_=t[:, idx * CH:(idx + 1) * CH])
                elif kind == "g":
                    nc.sync.dma_start(
                        out=sg_sb[:, idx * CH:(idx + 1) * CH],
                        in_=sg[:, idx * CH:(idx + 1) * CH])
                elif kind == "d":
                    ch = c8p.tile([128, CH], F8, tag="c8")
                    nc.sync.dma_start(out=ch, in_=sl8[:, idx * CH:(idx + 1) * CH])
                    return ch
                elif kind == "s":
                    ch = c16p.tile([128, CH], BF16, tag="c16")
                    nc.sync.dma_start(out=ch, in_=sl16[:, idx * CH:(idx + 1) * CH])
                    return ch
                else:  # half
                    ch = c16p.tile([128, 2048], BF16, tag="ch2")
                    lo = N_FULL * CH + idx * 2048
                    nc.sync.dma_start(out=ch, in_=sl16[:, lo:lo + 2048])
                    return ch
                return None

            def emit_sg_mms(n):
                for _ in range(n):
                    j = sg_pieces_done[0]
                    if j < 32:
                        sg_mm(j)
                        sg_pieces_done[0] += 1

            def emit_p_mms(n):
                for _ in range(n):
                    j = p_pieces_done[0]
                    if j < 32:
                        p_mm(j)
                        p_pieces_done[0] += 1

            scan_seen = 0
            g_staged = [False]
            fill_pe(WARM)  # warm PE while first loads land
            for kind, idx in LOAD_ORDER:
                ch = load(kind, idx)
                if kind == "t":
                    t_exp(idx)
                    if idx == 3:
                        # zloc = row-sums of zT on ACT (keeps DVE free);
                        # in-stream right after the t3 exp that completes zT
                        nc.scalar.activation(thr4, zT, COPY, bias=0.0,
                                             scale=1.0, accum_out=zloc)
                elif kind == "g":
                    vhat_q(idx)
                    emit_sg_mms(8)
                    sg_exp(idx)
                elif kind in ("s", "d", "h"):
                    fill_pe(FILL)
                    sl_piece_mm(ch, kind)
                    if kind == "s" and idx == 0:
                        # Z fold -> 1/Z -> p-colsum weights.  The matmul sits
                        # after s0's sl matmuls in the PE stream (zloc ready
                        # by then); the DVE recip/scale follow it in EMISSION
                        # order (deps are built from program order).  wqs is
                        # thus written ~15us before the first p matmul can
                        # execute (PE weight reads race mid-kernel writers).
                        nc.tensor.matmul(bankZ, gm, zloc, start=True,
                                         stop=True, skip_group_check=True)
                        fill_pe(3)
                        nc.vector.reciprocal(rz, bankZ)
                        nc.vector.tensor_scalar_mul(wqs, mq16, rz)
                        nc.scalar.dma_start(out=z_t[:, :], in_=zT)
                    if kind == "d":
                        dense_exp(ch, idx)
                        emit_p_mms(7)
                    else:
                        scan_unit(ch, half=(kind == "h"))
                        scan_seen += 1
                    if scan_seen == 5 and kind == "s":
                        nc.scalar.activation(
                            throw[:, 0:40], collect[:, 0:40], EXP,
                            bias=nbs_t, scale=1.0 / ts,
                            accum_out=wS[:, K_DENSE:K_DENSE + 1])
                    if slot_ctr[0] == 64:
                        stA = stage.tile([128, 512], F32, tag="st")
                        nc.vector.tensor_copy(out=stA, in_=bankA)
                        nc.gpsimd.dma_start(out=scols[:, 0:512], in_=stA)
                    if sg_pieces_done[0] == 32 and not g_staged[0]:
                        g_staged[0] = True
                        stG = stage.tile([128, 512], F32, tag="st")
                        nc.vector.tensor_copy(out=stG, in_=bankG)
                        nc.gpsimd.dma_start(out=gcols[:, :], in_=stG)
                        nc.gpsimd.dma_start(out=w_sg[:, :], in_=wG)

            emit_p_mms(32)
            nc.gpsimd.dma_start(out=v_t[:, :], in_=vT)

            # W batch 2 + tails
            nc.scalar.activation(
                throw[:, 0:ncol_collect - 40], collect[:, 40:ncol_collect],
                EXP, bias=nbs_t, scale=1.0 / ts,
                accum_out=wS[:, K_DENSE + 1:K_DENSE + 2])
            nc.scalar.dma_start(out=w_sl[:, :], in_=wS)

            stB = stage.tile([128, 512], F32, tag="st")
            nc.vector.tensor_copy(out=stB, in_=bankB)
            nc.gpsimd.dma_start(out=scols[:, 512:1024], in_=stB)
            stP = stage.tile([128, 512], F32, tag="st")
            nc.vector.tensor_copy(out=stP, in_=bankP)
            nc.gpsimd.dma_start(out=pcols[:, :], in_=stP)

    nc.compile()
    return nc


_NC_CACHE = {}


def _get_nc(ts, tt):
    key = (round(ts, 9), round(tt, 9))
    if key not in _NC_CACHE:
        _NC_CACHE[key] = build_nc(ts=ts, tt=tt)
    return _NC_CACHE[key]


def _merge(results, ts, bs_scaled):
    """Host-side exact merge of per-core device outputs (float64)."""
    slots = _slot_map()
    S = np.zeros(D, np.float64)
    P = np.zeros(D, np.float64)
    C = 0.0
    C_g = 0.0
    diag1 = 0.0
    healthy = True
    for r in results:
        sc = r["scols"].astype(np.float64)     # [128, 1024]
        for s, base in enumerate(slots):
            col = sc[s, 0:512] if s < 64 else sc[s - 64, 512:1024]
            S[base:base + 512] += col
        gc = r["gcols"].astype(np.float64)     # [128, 512]
        pc = r["pcols"].astype(np.float64)
        for j in range(32):
            for q in range(4):
                lo = q * DQ + j * 512
                S[lo:lo + 512] += gc[4 * j + q]
                P[lo:lo + 512] += pc[4 * j + q]
        w = r["w_sl"].astype(np.float64)
        wsum = w.sum(axis=1)
        healthy &= bool(np.isfinite(w).all() and (wsum > 0).all())
        C += (bs_scaled + np.log(np.maximum(wsum, 1e-300))).sum()
        wg = r["w_sg"].astype(np.float64)
        healthy &= bool(np.isfinite(wg).all() and (wg.sum(axis=1) > 0).all())
        lp = (bs_scaled + np.log(np.maximum(wg.sum(axis=1), 1e-300))).reshape(32, 4)
        mxg = lp.max(axis=1, keepdims=True)
        lse_g = mxg[:, 0] + np.log(np.exp(lp - mxg).sum(axis=1))
        C += lse_g.sum()
        C_g += lse_g.sum()
        v = r["v_t"].astype(np.float64).sum(axis=1).reshape(32, 4).sum(axis=1)
        z = r["z_t"].astype(np.float64).sum(axis=1).reshape(32, 4).sum(axis=1)
        healthy &= bool(np.isfinite(v).all() and np.isfinite(z).all()
                        and (z > 0).all())
        diag1 += (v / np.maximum(z, 1e-300)).sum() / ts
        healthy &= bool(np.isfinite(r["scols"]).all()
                        and np.isfinite(r["gcols"]).all()
                        and np.isfinite(r["pcols"]).all())

    cross = P @ S / ts - C * P.sum()
    diag = diag1 - C_g
    total = -cross + diag
    n_s = N_G + N_L
    n_loss_terms = N_T * n_s - min(N_T, n_s)
    loss = total / n_loss_terms
    healthy &= bool(np.isfinite(loss))
    return loss, healthy


def _numpy_loss(sg_full, sl_full, teacher, ts, tt):
    """Exact host fallback (never hit for sane input distributions)."""
    x = np.concatenate([sg_full, sl_full], axis=0).astype(np.float64) / ts
    lq = x - x.max(axis=1, keepdims=True)
    lq -= np.log(np.exp(lq).sum(axis=1, keepdims=True))
    y = teacher.astype(np.float64) / tt
    e = np.exp(y - y.max(axis=1, keepdims=True))
    p = e / e.sum(axis=1, keepdims=True)
    ce = -(p @ lq.T)
    n_t, n_s = ce.shape
    idx = np.arange(n_t)
    ce[idx, idx] = 0.0
    return ce.sum() / (n_t * n_s - min(n_t, n_s))


def kernel(out_student_global, out_student_local, out_teacher, center,
           temp_student, temp_teacher, cent_rate_m):
    out_student_global = np.asarray(out_student_global)
    out_student_local = np.asarray(out_student_local)
    out_teacher = np.asarray(out_teacher)
    center = np.asarray(center)
    ts = float(np.asarray(temp_student).reshape(-1)[0])
    tt = float(np.asarray(temp_teacher).reshape(-1)[0])

    teacher = out_teacher
    if np.any(center):
        teacher = out_teacher - center.reshape(1, -1).astype(np.float32)
    teacher = np.ascontiguousarray(teacher, dtype=np.float32)
    sg_full = np.ascontiguousarray(out_student_global, dtype=np.float32)
    sl_full = np.ascontiguousarray(out_student_local, dtype=np.float32)

    # safe exp bound for student rows: strided-sample max + margin
    smax = max(float(sl_full.ravel()[::257].max()),
               float(sg_full.ravel()[::257].max()))
    b_s = smax + 1.0
    nbs_col = np.full((128, 1), -b_s / ts, np.float32)
    # exact teacher row maxes (device exp bias); clamp so every bf16 e_t
    # value stays in the normal range (PE mishandles bf16 subnormals);
    # adds < 65536*e^-25 ~ 1e-6 relative spurious mass to Z
    tmax = teacher.max(axis=1)
    teacher = np.maximum(teacher, (tmax - 25.0 * tt)[:, None])

    nc = _get_nc(ts, tt)
    T_ROWS = N_T // N_CORES
    SG_ROWS = N_G // N_CORES
    SL_ROWS = N_L // N_CORES
    split = K_DENSE * CH
    in_maps = []
    for c in range(N_CORES):
        slc = sl_full[c * SL_ROWS:(c + 1) * SL_ROWS]
        ntm_c = (-np.repeat(tmax[c * T_ROWS:(c + 1) * T_ROWS], 4)
                 .reshape(128, 1) / tt).astype(np.float32)
        in_maps.append({
            "sl8": np.ascontiguousarray(slc[:, :split]).astype(NP_F8),
            "sl16": np.ascontiguousarray(slc[:, split:]).astype(NP_BF16),
            "sg": sg_full[c * SG_ROWS:(c + 1) * SG_ROWS]
                  .reshape(128, DQ).astype(NP_BF16),
            "t": teacher[c * T_ROWS:(c + 1) * T_ROWS]
                 .reshape(128, DQ).astype(NP_F8),
            "nbs": np.ascontiguousarray(
                np.concatenate([nbs_col, ntm_c], axis=1)).astype(np.float32),
        })
    res = run_bass_kernel_spmd(nc, in_maps, core_ids=list(range(N_CORES)))
    loss, healthy = _merge(res.results, ts, b_s / ts)
    if not healthy:
        loss = _numpy_loss(sg_full, sl_full, teacher, ts, tt)
    return np.float32(loss)


# revision 33
# speedup vs baseline: 1.3134x; 1.0871x over previous
"""DINO loss kernel for Trainium2 (8 NeuronCores, Bass/Tile) — v2.

Math (identical factorization to the fp32 baseline)
---------------------------------------------------
With q = log_softmax(student/ts) [Ns=1280, D=65536] and
p = softmax((teacher-center)/tt) [Nt=256, D]:

    loss = sum_{i != j} ( -sum_d p[i,d] q[j,d] ) / (Nt*Ns - Nt)
         = ( -(P.S/ts - C*sum(P)) + diag ) / (Nt*Ns - Nt)

    P[d] = teacher prob column sums          (device)
    S[d] = raw student logit column sums     (device)
    C    = sum_j logsumexp_j(x/ts)           (device partials, host log)
    diag = sum_i v_i/(ts*Z_i) - C_g          (v_i = sum_d e_t*sg, device)

v2: dtype-compressed transfers + top-8 logsumexp scan
-----------------------------------------------------
The fp32 baseline was DMA-bound at 48 MiB/core (~140us floor).  v2 ships
19.5 MiB/core: student_local as 5 fp8e4m3 chunks + 9 bf16 chunks + 4 bf16
half-chunks, student_global bf16, teacher fp8 (host-clamped to rowmax -
25*tt: PE mishandles bf16-subnormal e_t values).  Loss error ~5e-4 vs
the 2e-2 tolerance (validated in fp64 simulation): colsums and softmax
stats average the per-element rounding noise away.

ACT (1.2 GHz/col, dtype-blind) cannot exp everything under the ~57us DMA
floor, so row-logsumexp of the bf16 sl chunks is a DVE scan: split-half
bf16 max folds (2x mode, exact) -> InstMax top-8 per row -> ACT exps
just the 8 candidates/chunk.  At ts=0.1 the lse is top-few dominated;
rank-9+ within a folded chunk contributes < 1e-5 of the row sum.

Colsums pack vertically: sliding single-column (sl) / quad (sg, p) masks
route each 512-col piece's colsum into distinct PSUM *rows* of one
[128,512] bank via long start/stop accumulation chains, so each output
stream stages with ONE [128,512] DVE copy and retires with ONE Pool DMA.

Hardware quirks found on real trn2 (cost-model sim is blind to all):
  * TensorTensorReduce crashes the device (any dtype) — vhat is TT-mul
    (2x) + TT-add folds + short reduce instead.
  * TensorTensor is rejected on GPSIMD/Pool by codegen.
  * PE matmul weight reads do NOT wait for mid-kernel writers (DVE, ACT
    or DMA): weights must be written well before the consumer's position
    in the PE stream.  The p-colsum weights (wqs = quad mask * 1/Z) are
    therefore produced ~15us before the first p matmul, whose execution
    is structurally gated by the late-scheduled dense-chunk DMAs.
  * A dead PSUM bank absorbs PE keep-warm fillers (idle resets the
    clock ramp; cold-released matmuls cost 2-4x).
"""

import numpy as np
import ml_dtypes

import concourse.bass as bass
import concourse.bacc as bacc
import concourse.tile as tile
from concourse import mybir
from concourse.bass_utils import run_bass_kernel_spmd

F32 = mybir.dt.float32
BF16 = mybir.dt.bfloat16
F8 = mybir.dt.float8e4
AX = mybir.AxisListType
EXP = mybir.ActivationFunctionType.Exp
COPY = mybir.ActivationFunctionType.Copy
MAX = mybir.AluOpType.max
ADD = mybir.AluOpType.add
MUL = mybir.AluOpType.mult

NP_BF16 = ml_dtypes.bfloat16
NP_F8 = ml_dtypes.float8_e4m3

N_CORES = 8
D = 65536
N_T, N_G, N_L = 256, 256, 1024
CH = 4096                  # sl chunk cols
K_DENSE = 5                # fp8 chunks, exp'd densely on ACT
N_FULL = 9                 # bf16 scan chunks (fold3 + top8)
N_HALF = 4                 # trailing bf16 half-chunks (fold2 + top8)
DQ = D // 4                # sg/t cols after x4 row split

# DMA-arrival order.  Teacher first (longest dependency chain: exp -> Z ->
# 1/Z -> wqs weights), sg quarters early (vhat), dense fp8 chunks LATE:
# their PE matmuls precede the p-colsum matmuls in the PE stream, so the
# dense DMA arrival is the structural gate that keeps the p matmuls from
# racing the wqs weight write (PE weight reads don't wait for writers).
LOAD_ORDER = [
    ("t", 0), ("t", 1), ("t", 2), ("t", 3), ("g", 0), ("g", 1), ("s", 0),
    ("g", 2), ("s", 1), ("g", 3), ("s", 2), ("s", 3), ("s", 4), ("d", 0),
    ("s", 5), ("d", 1), ("s", 6), ("d", 2), ("s", 7), ("d", 3), ("s", 8),
    ("d", 4), ("h", 0), ("h", 1), ("h", 2), ("h", 3),
]

SL_ARRIVAL = [(k, i) for (k, i) in LOAD_ORDER if k in ("s", "d", "h")]


def _sl_piece_cols(kind, idx):
    """(global col base, n 512-windows) of an sl piece."""
    if kind == "d":
        return idx * CH, 8
    if kind == "s":
        return (K_DENSE + idx) * CH, 8
    return (K_DENSE + N_FULL) * CH + idx * 2048, 4


def _slot_map():
    """arrival-ordered window slots -> global sl col base (host decode)."""
    slots = []
    for kind, idx in SL_ARRIVAL:
        base, nw = _sl_piece_cols(kind, idx)
        for w in range(nw):
            slots.append(base + w * 512)
    assert len(slots) == 128
    return slots


def _masks_np():
    # sliding single-column mask: msl[p, c] = 1 iff c == 128
    msl = np.zeros((128, 256), np.float32)
    msl[:, 128] = 1.0
    # sliding quad mask: mq[p, c] = 1 iff c - 128 == p % 4
    mq = np.zeros((128, 260), np.float32)
    for p in range(128):
        mq[p, 128 + p % 4] = 1.0
    # Z-fold gmask: g[p, m] = 1 iff p//4 == m//4 (fold + broadcast in one mm)
    gm = (np.arange(128)[:, None] // 4 == np.arange(128)[None, :] // 4)
    return msl, mq, gm.astype(np.float32)


def build_nc(ts=0.1, tt=0.04, FILL=0, WARM=8, skip=()):
    nc = bacc.Bacc()
    sl16 = nc.dram_tensor("sl16", [128, (N_FULL * CH) + N_HALF * 2048], BF16,
                          kind="ExternalInput")
    sl8 = nc.dram_tensor("sl8", [128, K_DENSE * CH], F8, kind="ExternalInput")
    sg = nc.dram_tensor("sg", [128, DQ], BF16, kind="ExternalInput")
    t = nc.dram_tensor("t", [128, DQ], F8, kind="ExternalInput")
    nbs = nc.dram_tensor("nbs", [128, 2], F32, kind="ExternalInput")

    msl_np, mq_np, gm_np = _masks_np()
    msl16_d = nc.inline_tensor(np.ascontiguousarray(msl_np.astype(NP_BF16)), name="msl16")
    msl8_d = nc.inline_tensor(np.ascontiguousarray(msl_np.astype(NP_F8)), name="msl8")
    mq16_d = nc.inline_tensor(np.ascontiguousarray(mq_np.astype(NP_BF16)), name="mq16")
    gm_d = nc.inline_tensor(np.ascontiguousarray(gm_np), name="gmf32")

    scols = nc.dram_tensor("scols", [128, 1024], F32, kind="ExternalOutput")
    gcols = nc.dram_tensor("gcols", [128, 512], F32, kind="ExternalOutput")
    pcols = nc.dram_tensor("pcols", [128, 512], F32, kind="ExternalOutput")
    w_sl = nc.dram_tensor("w_sl", [128, K_DENSE + 2], F32, kind="ExternalOutput")
    w_sg = nc.dram_tensor("w_sg", [128, 4], F32, kind="ExternalOutput")
    z_t = nc.dram_tensor("z_t", [128, 4], F32, kind="ExternalOutput")
    v_t = nc.dram_tensor("v_t", [128, 4], F32, kind="ExternalOutput")

    n_scan_units = N_FULL + N_HALF
    ncol_collect = 8 * n_scan_units  # 104

    with tile.TileContext(nc) as tc:
        with (
            tc.tile_pool(name="singles", bufs=1) as singles,
            tc.tile_pool(name="big", bufs=1) as big,
            tc.tile_pool(name="c16", bufs=4) as c16p,
            tc.tile_pool(name="c8", bufs=3) as c8p,
            tc.tile_pool(name="l1p", bufs=2) as l1p,
            tc.tile_pool(name="l2p", bufs=2) as l2p,
            tc.tile_pool(name="l3p", bufs=2) as l3p,
            tc.tile_pool(name="stage", bufs=2) as stage,
            tc.tile_pool(name="psA", bufs=1, space="PSUM") as psA,
            tc.tile_pool(name="psB", bufs=1, space="PSUM") as psB,
            tc.tile_pool(name="psG", bufs=1, space="PSUM") as psG,
            tc.tile_pool(name="psP", bufs=1, space="PSUM") as psP,
            tc.tile_pool(name="psZ", bufs=1, space="PSUM") as psZ,
            tc.tile_pool(name="psF", bufs=1, space="PSUM") as psF,
        ):
            # ---- t=0: tiny loads on the ACT HWDGE queue ----
            msl16 = singles.tile([128, 256], BF16)
            nc.scalar.dma_start(out=msl16, in_=msl16_d[:, :])
            msl8 = singles.tile([128, 256], F8)
            nc.scalar.dma_start(out=msl8, in_=msl8_d[:, :])
            mq16 = singles.tile([128, 260], BF16)
            nc.scalar.dma_start(out=mq16, in_=mq16_d[:, :])
            gm = singles.tile([128, 128], F32)
            nc.scalar.dma_start(out=gm, in_=gm_d[:, :])
            nb = singles.tile([128, 2], F32)
            nc.scalar.dma_start(out=nb, in_=nbs[:, :])
            nbs_t = nb[:, 0:1]
            ntm_t = nb[:, 1:2]

            # resident SBUF tensors
            t_sb = big.tile([128, DQ], F8)
            sg_sb = big.tile([128, DQ], BF16)
            e_t = big.tile([128, DQ], BF16)
            throw = big.tile([128, 2 * CH], BF16)  # ACT exp target (dead)
            thr4 = big.tile([128, 4], F32)
            vm = big.tile([128, CH], BF16)         # vhat mul out
            va = big.tile([128, CH // 2], BF16)
            vb = big.tile([128, CH // 4], BF16)
            vc = big.tile([128, CH // 8], BF16)
            collect = big.tile([128, ncol_collect], BF16)
            frhs = big.tile([128, 512], BF16)      # filler rhs
            nc.vector.memset(frhs, 0.0)

            # stats
            wS = big.tile([128, K_DENSE + 2], F32)
            wG = big.tile([128, 4], F32)
            zT = big.tile([128, 4], F32)
            vT = big.tile([128, 4], F32)
            zloc = big.tile([128, 1], F32)
            rz = big.tile([128, 1], F32)
            wqs = big.tile([128, 260], BF16)

            # PSUM banks
            bankA = psA.tile([128, 512], F32)
            bankB = psB.tile([128, 512], F32)
            bankG = psG.tile([128, 512], F32)
            bankP = psP.tile([128, 512], F32)
            bankZ = psZ.tile([128, 1], F32)
            bankF = psF.tile([128, 512], F32)      # filler sink, never read

            def fill_pe(n):
                for _ in range(n):
                    nc.tensor.matmul(bankF, msl16[:, 0:128], frhs,
                                     start=True, stop=True,
                                     skip_group_check=True)

            # ---- sl window colsums (vertical packing) ----
            slot_ctr = [0]

            def sl_piece_mm(ch_tile, kind):
                mask = msl8 if kind == "d" else msl16
                nw = ch_tile.shape[-1] // 512
                for w in range(nw):
                    s = slot_ctr[0]
                    bank = bankA if s < 64 else bankB
                    r = s % 64
                    nc.tensor.matmul(
                        bank, mask[:, (128 - r):(256 - r)],
                        ch_tile[:, w * 512:(w + 1) * 512],
                        start=(r == 0), stop=(r == 63),
                        skip_group_check=True,
                    )
                    slot_ctr[0] += 1

            # ---- scan unit: fold + top8 into collect ----
            unit_ctr = [0]

            def scan_unit(ch_tile, half):
                u = unit_ctr[0]
                if not half:
                    a1 = l1p.tile([128, 2048], BF16, tag="l1")
                    nc.vector.tensor_tensor(out=a1, in0=ch_tile[:, 0:2048],
                                            in1=ch_tile[:, 2048:4096], op=MAX)
                else:
                    a1 = ch_tile
                a2 = l2p.tile([128, 1024], BF16, tag="l2")
                nc.vector.tensor_tensor(out=a2, in0=a1[:, 0:1024],
                                        in1=a1[:, 1024:2048], op=MAX)
                nc.vector.max(collect[:, 8 * u:8 * u + 8], a2)
                unit_ctr[0] += 1

            # ---- vhat quarter: mul + 3 fold-adds + short reduce ----
            def vhat_q(q):
                lo = q * CH
                nc.vector.tensor_tensor(out=vm, in0=e_t[:, lo:lo + CH],
                                        in1=sg_sb[:, lo:lo + CH], op=MUL)
                nc.vector.tensor_tensor(out=va, in0=vm[:, 0:2048],
                                        in1=vm[:, 2048:4096], op=ADD)
                nc.vector.tensor_tensor(out=vb, in0=va[:, 0:1024],
                                        in1=va[:, 1024:2048], op=ADD)
                nc.vector.tensor_tensor(out=vc, in0=vb[:, 0:512],
                                        in1=vb[:, 512:1024], op=ADD)
                nc.vector.reduce_sum(vT[:, q:q + 1], vc, axis=AX.X)

            def t_exp(q):
                nc.scalar.activation(
                    e_t[:, q * CH:(q + 1) * CH], t_sb[:, q * CH:(q + 1) * CH],
                    EXP, bias=ntm_t, scale=1.0 / tt,
                    accum_out=zT[:, q:q + 1],
                )

            def sg_exp(q):
                nc.scalar.activation(
                    throw[:, 0:CH], sg_sb[:, q * CH:(q + 1) * CH],
                    EXP, bias=nbs_t, scale=1.0 / ts,
                    accum_out=wG[:, q:q + 1],
                )

            def dense_exp(ch_tile, k):
                nc.scalar.activation(
                    throw[:, 0:CH], ch_tile, EXP, bias=nbs_t, scale=1.0 / ts,
                    accum_out=wS[:, k:k + 1],
                )

            def sg_mm(j):
                nc.tensor.matmul(
                    bankG, mq16[:, (128 - 4 * j):(256 - 4 * j)],
                    sg_sb[:, j * 512:(j + 1) * 512],
                    start=(j == 0), stop=(j == 31), skip_group_check=True,
                )

            def p_mm(j):
                nc.tensor.matmul(
                    bankP, wqs[:, (128 - 4 * j):(256 - 4 * j)],
                    e_t[:, j * 512:(j + 1) * 512],
                    start=(j == 0), stop=(j == 31), skip_group_check=True,
                )

            # ================= the weave =================
            sg_pieces_done = [0]
            p_pieces_done = [0]

            def load(kind, idx):
                if kind == "t":
                    nc.sync.dma_start(
                        out=t_sb[:, idx * CH:(idx + 1) * CH],
                        in_=t[:, idx * CH:(idx + 1) * CH])
                elif kind == "g":
                    nc.sync.dma_start(
                        out=sg_sb[:, idx * CH:(idx + 1) * CH],
                        in_=sg[:, idx * CH:(idx + 1) * CH])
                elif kind == "d":
                    ch = c8p.tile([128, CH], F8, tag="c8")
                    nc.sync.dma_start(out=ch, in_=sl8[:, idx * CH:(idx + 1) * CH])
                    return ch
                elif kind == "s":
                    ch = c16p.tile([128, CH], BF16, tag="c16")
                    nc.sync.dma_start(out=ch, in_=sl16[:, idx * CH:(idx + 1) * CH])
                    return ch
                else:  # half
                    ch = c16p.tile([128, 2048], BF16, tag="ch2")
                    lo = N_FULL * CH + idx * 2048
                    nc.sync.dma_start(out=ch, in_=sl16[:, lo:lo + 2048])
                    return ch
                return None

            def emit_sg_mms(n):
                for _ in range(n):
                    j = sg_pieces_done[0]
                    if j < 32:
                        sg_mm(j)
                        sg_pieces_done[0] += 1

            def emit_p_mms(n):
                for _ in range(n):
                    j = p_pieces_done[0]
                    if j < 32:
                        p_mm(j)
                        p_pieces_done[0] += 1

            scan_seen = 0
            g_staged = [False]
            fill_pe(WARM)  # warm PE while first loads land
            for kind, idx in LOAD_ORDER:
                ch = load(kind, idx)
                if kind == "t":
                    t_exp(idx)
                    if idx == 3:
                        # zloc = row-sums of zT on ACT (keeps DVE free);
                        # in-stream right after the t3 exp that completes zT
                        nc.scalar.activation(thr4, zT, COPY, bias=0.0,
                                             scale=1.0, accum_out=zloc)
                elif kind == "g":
                    if "vhat" not in skip:
                        vhat_q(idx)
                    emit_sg_mms(8)
                    if "gexp" not in skip:
                        sg_exp(idx)
                elif kind in ("s", "d", "h"):
                    fill_pe(FILL)
                    if "slmm" not in skip:
                        sl_piece_mm(ch, kind)
                    else:
                        slot_ctr[0] += ch.shape[-1] // 512
                    if kind == "s" and idx == 0:
                        # Z fold -> 1/Z -> p-colsum weights.  The matmul sits
                        # after s0's sl matmuls in the PE stream (zloc ready
                        # by then); the DVE recip/scale follow it in EMISSION
                        # order (deps are built from program order).  wqs is
                        # thus written ~15us before the first p matmul can
                        # execute (PE weight reads race mid-kernel writers).
                        nc.tensor.matmul(bankZ, gm, zloc, start=True,
                                         stop=True, skip_group_check=True)
                        fill_pe(3)
                        nc.vector.reciprocal(rz, bankZ)
                        nc.vector.tensor_scalar_mul(wqs, mq16, rz)
                        nc.scalar.dma_start(out=z_t[:, :], in_=zT)
                    if kind == "d":
                        if "dexp" not in skip:
                            dense_exp(ch, idx)
                        emit_p_mms(0 if "pmm" in skip else 7)
                    else:
                        if "scan" not in skip:
                            scan_unit(ch, half=(kind == "h"))
                        scan_seen += 1
                    if scan_seen == 5 and kind == "s":
                        nc.scalar.activation(
                            throw[:, 0:40], collect[:, 0:40], EXP,
                            bias=nbs_t, scale=1.0 / ts,
                            accum_out=wS[:, K_DENSE:K_DENSE + 1])
                    if slot_ctr[0] == 64:
                        stA = stage.tile([128, 512], F32, tag="st")
                        nc.vector.tensor_copy(out=stA, in_=bankA)
                        nc.gpsimd.dma_start(out=scols[:, 0:512], in_=stA)
                    if sg_pieces_done[0] == 32 and not g_staged[0]:
                        g_staged[0] = True
                        stG = stage.tile([128, 512], F32, tag="st")
                        nc.vector.tensor_copy(out=stG, in_=bankG)
                        nc.gpsimd.dma_start(out=gcols[:, :], in_=stG)
                        nc.gpsimd.dma_start(out=w_sg[:, :], in_=wG)

            emit_p_mms(0 if "pmm" in skip else 32)
            nc.gpsimd.dma_start(out=v_t[:, :], in_=vT)

            # W batch 2 + tails
            nc.scalar.activation(
                throw[:, 0:ncol_collect - 40], collect[:, 40:ncol_collect],
                EXP, bias=nbs_t, scale=1.0 / ts,
                accum_out=wS[:, K_DENSE + 1:K_DENSE + 2])
            nc.scalar.dma_start(out=w_sl[:, :], in_=wS)

            stB = stage.tile([128, 512], F32, tag="st")
            nc.vector.tensor_copy(out=stB, in_=bankB)
            nc.gpsimd.dma_start(out=scols[:, 512:1024], in_=stB)
            stP = stage.tile([128, 512], F32, tag="st")
            nc.vector.tensor_copy(out=stP, in_=bankP)
            nc.gpsimd.dma_start(out=pcols[:, :], in_=stP)

    nc.compile()
    return nc


_NC_CACHE = {}


def _get_nc(ts, tt):
    key = (round(ts, 9), round(tt, 9))
    if key not in _NC_CACHE:
        _NC_CACHE[key] = build_nc(ts=ts, tt=tt)
    return _NC_CACHE[key]


def _merge(results, ts, bs_scaled):
    """Host-side exact merge of per-core device outputs (float64)."""
    slots = _slot_map()
    S = np.zeros(D, np.float64)
    P = np.zeros(D, np.float64)
    C = 0.0
    C_g = 0.0
    diag1 = 0.0
    healthy = True
    for r in results:
        sc = r["scols"].astype(np.float64)     # [128, 1024]
        for s, base in enumerate(slots):
            col = sc[s, 0:512] if s < 64 else sc[s - 64, 512:1024]
            S[base:base + 512] += col
        gc = r["gcols"].astype(np.float64)     # [128, 512]
        pc = r["pcols"].astype(np.float64)
        for j in range(32):
            for q in range(4):
                lo = q * DQ + j * 512
                S[lo:lo + 512] += gc[4 * j + q]
                P[lo:lo + 512] += pc[4 * j + q]
        w = r["w_sl"].astype(np.float64)
        wsum = w.sum(axis=1)
        healthy &= bool(np.isfinite(w).all() and (wsum > 0).all())
        C += (bs_scaled + np.log(np.maximum(wsum, 1e-300))).sum()
        wg = r["w_sg"].astype(np.float64)
        healthy &= bool(np.isfinite(wg).all() and (wg.sum(axis=1) > 0).all())
        lp = (bs_scaled + np.log(np.maximum(wg.sum(axis=1), 1e-300))).reshape(32, 4)
        mxg = lp.max(axis=1, keepdims=True)
        lse_g = mxg[:, 0] + np.log(np.exp(lp - mxg).sum(axis=1))
        C += lse_g.sum()
        C_g += lse_g.sum()
        v = r["v_t"].astype(np.float64).sum(axis=1).reshape(32, 4).sum(axis=1)
        z = r["z_t"].astype(np.float64).sum(axis=1).reshape(32, 4).sum(axis=1)
        healthy &= bool(np.isfinite(v).all() and np.isfinite(z).all()
                        and (z > 0).all())
        diag1 += (v / np.maximum(z, 1e-300)).sum() / ts
        healthy &= bool(np.isfinite(r["scols"]).all()
                        and np.isfinite(r["gcols"]).all()
                        and np.isfinite(r["pcols"]).all())

    cross = P @ S / ts - C * P.sum()
    diag = diag1 - C_g
    total = -cross + diag
    n_s = N_G + N_L
    n_loss_terms = N_T * n_s - min(N_T, n_s)
    loss = total / n_loss_terms
    healthy &= bool(np.isfinite(loss))
    return loss, healthy


def _numpy_loss(sg_full, sl_full, teacher, ts, tt):
    """Exact host fallback (never hit for sane input distributions)."""
    x = np.concatenate([sg_full, sl_full], axis=0).astype(np.float64) / ts
    lq = x - x.max(axis=1, keepdims=True)
    lq -= np.log(np.exp(lq).sum(axis=1, keepdims=True))
    y = teacher.astype(np.float64) / tt
    e = np.exp(y - y.max(axis=1, keepdims=True))
    p = e / e.sum(axis=1, keepdims=True)
    ce = -(p @ lq.T)
    n_t, n_s = ce.shape
    idx = np.arange(n_t)
    ce[idx, idx] = 0.0
    return ce.sum() / (n_t * n_s - min(n_t, n_s))


def kernel(out_student_global, out_student_local, out_teacher, center,
           temp_student, temp_teacher, cent_rate_m):
    out_student_global = np.asarray(out_student_global)
    out_student_local = np.asarray(out_student_local)
    out_teacher = np.asarray(out_teacher)
    center = np.asarray(center)
    ts = float(np.asarray(temp_student).reshape(-1)[0])
    tt = float(np.asarray(temp_teacher).reshape(-1)[0])

    teacher = out_teacher
    if np.any(center):
        teacher = out_teacher - center.reshape(1, -1).astype(np.float32)
    teacher = np.ascontiguousarray(teacher, dtype=np.float32)
    sg_full = np.ascontiguousarray(out_student_global, dtype=np.float32)
    sl_full = np.ascontiguousarray(out_student_local, dtype=np.float32)

    # safe exp bound for student rows: strided-sample max + margin
    smax = max(float(sl_full.ravel()[::257].max()),
               float(sg_full.ravel()[::257].max()))
    b_s = smax + 1.0
    nbs_col = np.full((128, 1), -b_s / ts, np.float32)
    # exact teacher row maxes (device exp bias); clamp so every bf16 e_t
    # value stays in the normal range (PE mishandles bf16 subnormals);
    # adds < 65536*e^-25 ~ 1e-6 relative spurious mass to Z
    tmax = teacher.max(axis=1)
    teacher = np.maximum(teacher, (tmax - 25.0 * tt)[:, None])

    nc = _get_nc(ts, tt)
    T_ROWS = N_T // N_CORES
    SG_ROWS = N_G // N_CORES
    SL_ROWS = N_L // N_CORES
    split = K_DENSE * CH
    in_maps = []
    for c in range(N_CORES):
        slc = sl_full[c * SL_ROWS:(c + 1) * SL_ROWS]
        ntm_c = (-np.repeat(tmax[c * T_ROWS:(c + 1) * T_ROWS], 4)
                 .reshape(128, 1) / tt).astype(np.float32)
        in_maps.append({
            "sl8": np.ascontiguousarray(slc[:, :split]).astype(NP_F8),
            "sl16": np.ascontiguousarray(slc[:, split:]).astype(NP_BF16),
            "sg": sg_full[c * SG_ROWS:(c + 1) * SG_ROWS]
                  .reshape(128, DQ).astype(NP_BF16),
            "t": teacher[c * T_ROWS:(c + 1) * T_ROWS]
                 .reshape(128, DQ).astype(NP_F8),
            "nbs": np.ascontiguousarray(
                np.concatenate([nbs_col, ntm_c], axis=1)).astype(np.float32),
        })
    res = run_bass_kernel_spmd(nc, in_maps, core_ids=list(range(N_CORES)))
    loss, healthy = _merge(res.results, ts, b_s / ts)
    if not healthy:
        loss = _numpy_loss(sg_full, sl_full, teacher, ts, tt)
    return np.float32(loss)
